# revision 66
# baseline (speedup 1.0000x reference)
"""Deformable-attention encoder layer (single level) on 8 Trainium2 cores.

Data-parallel over batch: B=16 -> 2 images per NeuronCore. Each core runs the
full layer for its 2 images; no collectives.

MSDA sampling strategy: the value projection is computed 4 times with the src
rows shifted by (0, 1, 64, 65), giving, for every spatial cell e=(y,x), the
full 2x2 bilinear patch per head laid out [d(32), c(4)] (128 bf16 = 256B).
Patches go to a DRAM table indexed by (e, head) and are fetched with gpsimd
dma_gather in chunks of 1024 indices (>=2048 indices per gather overflows the
SWDGE descriptor ring on this runtime and wedges the device). The int16 index
tile must be in dma_gather's wrapped layout (index i at partition i%16, column
i//16, replicated across the 8 Q7 core groups), produced by a PE transpose +
a shuffled DRAM round trip.

v2 changes vs the 1.30ms baseline (cost-model-guided; DVE was 78% busy at
1.37ms simulated, this version simulates at 0.79ms with DVE/DMA balanced
at ~62% each):
- pos@[W_off|W_attn] + ref*64-0.5 + b_off|b_attn precomputed on host into
  a 96-float "posoa" stream: kills the posT loads, 2 of 4 q-proj matmuls
  and the on-device reference-point math at full fp32 precision.
- residual adds run on the PE: src / x1 chunks enter the out-proj / FFN2
  PSUM accumulation via [0|I|0] "idstack" matmuls from the resident
  srcT/x1T tiles; the fp32 src stream and two DVE adds are gone.
- the 16-term bilinear corner sum runs as 4 strided halving tensor_tensor
  adds (mostly 2x DVE mode) instead of a 1x TensorReduce + 3 tree adds.
- phase-A index math is batched over G=4 tiles per instruction (amortizes
  the 58-cycle DVE instruction overhead).
- PSUM->SBUF moves (patch rows, gT, x1T) run on the Activation engine.
- zero biases / unit LN gains (per the generating spec) skip their ops;
  build_nc checks the actual input values and only emits the adds when
  nonzero (from inline constants), so generality is preserved.
- floor() via int32 round trip of x+1023.5 (round-to-nearest; a flip at
  exact integers lands the full weight on the equivalent other corner).
  NOTE: the fp32 +1.5*2^23 magic-number floor is numerically broken on
  real DVE hardware (0.26 rel err) despite being architecturally sound -
  keep the int cast.
- output stream is bf16 (host upcasts): halves the final store DMA for
  ~2e-3 extra mean-relative error against a 2e-2 budget.
- emission is software-pipelined A(0) | B(0)+A(1) | B(1) with per-batch
  DRAM tables: batch 1's projections/table writes overlap batch 0's
  gather+attention phase. Engine SEQ queues are in-order, so overlap
  exists only if instruction emission interleaves.

Tried and rejected (cost-model evidence): 2048-idx gathers w/ enlarged
SWDGE ring (Pool halves, longer exclusive DMA holds cost more); G=8
batching; table-first prologue and per-group idx/w4 tiles (extra DMAs +
dep edges cost ~30us; B(0) is gated on the full table either way); deeper
work/mac pools; ppt/ppv/ppf rebalancing.
"""

import sys
from contextlib import ExitStack

import numpy as np

sys.path.insert(0, "/opt/trn_rl_repo")

import concourse.bass as bass
import concourse.bacc as bacc
import concourse.mybir as mybir
import concourse.tile as tile
from concourse.bass_utils import run_bass_kernel_spmd

F32 = mybir.dt.float32
BF16 = mybir.dt.bfloat16
I16 = mybir.dt.int16
I32 = mybir.dt.int32
AF = mybir.ActivationFunctionType
OP = mybir.AluOpType

B, LQ, C = 16, 4096, 256
NH, NP, DH = 8, 4, 32
HS, WS = 64, 64
DFF = 1024
NCORES = 8
BPC = B // NCORES          # batches per core
P = 128
NT = LQ // P               # q-tiles per batch
EPAD = 128                 # srcT column padding for shifted windows

_CACHE = {}


def _bcast_ap(handle_or_ap, n_free):
    """DRAM [n] -> AP [128, n] replicated across partitions."""
    ap = handle_or_ap[:] if not isinstance(handle_or_ap, bass.AP) else handle_or_ap
    return bass.AP(tensor=ap.tensor, offset=ap.offset, ap=[[0, P], [1, n_free]])


def _ap(t, offset, dims):
    return bass.AP(tensor=t.tensor, offset=t.offset + offset, ap=dims)


def build_nc(consts_in):
    """consts_in: dict with the (host-checked) bias / LN parameter values so
    ops for all-zero biases / unit gains are skipped."""
    nz = {k: bool(np.any(np.asarray(v) != 0)) for k, v in consts_in.items()
          if k in ("b_val", "b_out", "b2", "ln1_b", "ln2_b")}
    nz["ln1_g"] = bool(np.any(np.asarray(consts_in["ln1_g"]) != 1))
    nz["ln2_g"] = bool(np.any(np.asarray(consts_in["ln2_g"]) != 1))

    nc = bacc.Bacc("TRN2")

    # host-pretransposed [batch*C, LQ] copy feeding the matmul lhsT layout
    srcT_d = nc.dram_tensor("srcT", [BPC * C, LQ], BF16, kind="ExternalInput")
    # host-precomputed pos@[W_off|W_attn] + [ref*64-0.5 | 0] + [b_off|b_attn]
    posoa_d = nc.dram_tensor("posoa", [BPC * LQ, 96], F32, kind="ExternalInput")
    w_off_d = nc.dram_tensor("W_off", [C, 64], BF16, kind="ExternalInput")
    w_attn_d = nc.dram_tensor("W_attn", [C, 32], BF16, kind="ExternalInput")
    w_val_d = nc.dram_tensor("W_val", [C, C], BF16, kind="ExternalInput")
    w_out_d = nc.dram_tensor("W_out", [C, C + 1], BF16, kind="ExternalInput")
    w1_d = nc.dram_tensor("W1", [C, DFF], BF16, kind="ExternalInput")
    b1_d = nc.dram_tensor("b1", [DFF], F32, kind="ExternalInput")
    w2_d = nc.dram_tensor("W2", [DFF, C + 1], BF16, kind="ExternalInput")
    out_d = nc.dram_tensor("out", [BPC * LQ, C], BF16, kind="ExternalOutput")

    # patch table: row (e*8 + h), one tensor PER BATCH so phase A of batch
    # b+1 (table writes) doesn't serialize behind batch b's gathers
    h2_ds = [nc.dram_tensor(f"h2tab{b}", [LQ * NH, 4 * DH], BF16, kind="Internal")
             for b in range(BPC)]
    # per-tile gather indices in dma_gather's wrapped layout, int16
    idxshuf_ds = [nc.dram_tensor(f"idxshuf{b}", [NT * P * 32], I16, kind="Internal")
                  for b in range(BPC)]

    import ml_dtypes
    ident_d = nc.inline_tensor(np.eye(P, dtype=np.float32), "identconst")
    identb_d = nc.inline_tensor(
        np.eye(P, dtype=np.float32).astype(ml_dtypes.bfloat16), "identbconst")
    # idstack[k] = [0 .. I .. 0]: srcT_k.T @ idstack[k] drops the src chunk
    # into column block k of an accumulating PSUM tile (residual adds on PE)
    idstack_np = np.zeros((P, 2, C + 1), dtype=np.float32)
    idstack_np[:, 0, 0:P] = np.eye(P)
    idstack_np[:, 1, P:C] = np.eye(P)
    idstack_np[:, :, C] = 1.0 / C            # mean column (2^-8, bf16-exact)
    idstack_d = nc.inline_tensor(
        idstack_np.reshape(P, 2 * (C + 1)).astype(ml_dtypes.bfloat16),
        "idstackconst")
    # per-(h,p) head index constant, replicated over partitions
    hc = np.tile(np.repeat(np.arange(NH, dtype=np.float32), NP)[None, :], (P, 1))
    hc_d = nc.inline_tensor(hc, "hconst")

    def _maybe_const(name, vals, n):
        """[128, n]-replicated inline const for a nonzero bias vector."""
        arr = np.tile(np.asarray(vals, dtype=np.float32)[None, :], (P, 1))
        return nc.inline_tensor(arr, name)

    with ExitStack() as ctx:
        tc = ctx.enter_context(tile.TileContext(nc))
        consts = ctx.enter_context(tc.tile_pool(name="consts", bufs=1))
        persist = ctx.enter_context(tc.tile_pool(name="persist", bufs=1))
        work = ctx.enter_context(tc.tile_pool(name="work", bufs=2))
        io = ctx.enter_context(tc.tile_pool(name="io", bufs=4))
        achain = ctx.enter_context(tc.tile_pool(name="achain", bufs=1))
        patches_p = ctx.enter_context(tc.tile_pool(name="patches", bufs=2))
        ppt = ctx.enter_context(tc.tile_pool(name="ppt", bufs=2, space="PSUM"))
        pps = ctx.enter_context(tc.tile_pool(name="pps", bufs=2, space="PSUM"))
        ppv = ctx.enter_context(tc.tile_pool(name="ppv", bufs=2, space="PSUM"))
        ppf = ctx.enter_context(tc.tile_pool(name="ppf", bufs=2, space="PSUM"))

        # ---- constants / weights into SBUF ----
        ident = consts.tile([P, P], F32)
        nc.sync.dma_start(out=ident[:], in_=ident_d[:, :])
        identb = consts.tile([P, P], BF16)
        nc.sync.dma_start(out=identb[:], in_=identb_d[:, :])
        idstack = consts.tile([P, 2, C + 1], BF16)
        nc.sync.dma_start(out=idstack[:].rearrange("p k c -> p (k c)"),
                          in_=idstack_d[:, :])
        hcst = consts.tile([P, 32], F32)
        nc.sync.dma_start(out=hcst[:], in_=hc_d[:, :])
        epsb = consts.tile([P, 1], F32)
        nc.vector.memset(epsb[:], 1e-5)

        wb = consts.tile([P, 2, 96], BF16)          # W_off|W_attn, 2 k-tiles
        for k in range(2):
            nc.sync.dma_start(out=wb[:, k, 0:64], in_=w_off_d[128 * k:128 * (k + 1), :])
            nc.sync.dma_start(out=wb[:, k, 64:96], in_=w_attn_d[128 * k:128 * (k + 1), :])
        wval = consts.tile([P, 2, C], BF16)
        wout = consts.tile([P, 2, C + 1], BF16)
        for k in range(2):
            nc.sync.dma_start(out=wval[:, k, :], in_=w_val_d[128 * k:128 * (k + 1), :])
            nc.sync.dma_start(out=wout[:, k, :], in_=w_out_d[128 * k:128 * (k + 1), :])
        w1 = consts.tile([P, 2, DFF], BF16)
        for k in range(2):
            nc.sync.dma_start(out=w1[:, k, :], in_=w1_d[128 * k:128 * (k + 1), :])
        w2 = consts.tile([P, 8, C + 1], BF16)
        for j in range(8):
            nc.sync.dma_start(out=w2[:, j, :], in_=w2_d[128 * j:128 * (j + 1), :])

        # b1 transposed: [128, 8] with b1t[p, j] = b1[128j + p] (free via ACT bias)
        b1t = consts.tile([P, 8], F32)
        b1_ap = bass.AP(tensor=b1_d[:].tensor, offset=0, ap=[[1, P], [P, 8]])
        nc.sync.dma_start(out=b1t[:], in_=b1_ap)

        # optional nonzero bias / non-unit LN constants
        cext = {}
        for name, width in (("b_val", C), ("b_out", C), ("b2", C),
                            ("ln1_g", C), ("ln1_b", C),
                            ("ln2_g", C), ("ln2_b", C)):
            if nz.get(name):
                d = _maybe_const(name + "c", np.asarray(consts_in[name]), width)
                tl = consts.tile([P, width], F32)
                nc.sync.dma_start(out=tl[:], in_=d[:, :])
                cext[name] = tl

        G = 4                    # phase-A tiles batched per index-math group
        NG = NT // G
        srcTs, idx_alls, w4_alls = {}, {}, {}

        def setup_batch(b):
            srcTs[b] = persist.tile([P, 2, LQ + EPAD], BF16, tag=f"srcT{b}", name=f"srcT{b}")
            for k in range(2):
                nc.vector.memset(srcTs[b][:, k, LQ:LQ + EPAD], 0.0)
                nc.sync.dma_start(
                    out=srcTs[b][:, k, 0:LQ],
                    in_=srcT_d[b * 2 * P + k * P:b * 2 * P + (k + 1) * P, :])
            idx_alls[b] = persist.tile([P, NT, 256], I16, tag=f"idx_all{b}", name=f"idx_all{b}")
            w4_alls[b] = persist.tile([P, NT, P], BF16, tag=f"w4_all{b}", name=f"w4_all{b}")

        def emit_A_group(b, tg):
            """Index math / weights / patch table for tiles tg*G..tg*G+G-1."""
            srcT, w4_all = srcTs[b], w4_alls[b]
            t0g = tg * G
            r0g = b * LQ + t0g * P
            # host pre-grouped layout [b, tg, p, g, 96]: one contiguous
            # 1536B chunk per partition (no small-transfer DMA penalty)
            posoa = io.tile([P, G, 96], F32, tag="posoa")
            nc.sync.dma_start(
                out=posoa[:],
                in_=_ap(posoa_d[:], (b * NG + tg) * P * G * 96,
                        [[G * 96, P], [1, G * 96]]))

            # oa projections per tile; px/ain collected into group tiles
            px = achain.tile([P, G, 64], F32, tag="px")
            ain = achain.tile([P, G, 32], F32, tag="ain")
            for g in range(G):
                t = t0g + g
                oa = ppv.tile([P, 96], F32, tag="pv")
                nc.tensor.matmul(oa[:, 0:96], srcT[:, 0, t * P:(t + 1) * P],
                                 wb[:, 0, :], start=True, stop=False)
                nc.tensor.matmul(oa[:, 0:96], srcT[:, 1, t * P:(t + 1) * P],
                                 wb[:, 1, :], start=False, stop=True)
                nc.vector.tensor_tensor(out=px[:, g], in0=oa[:, 0:64],
                                        in1=posoa[:, g, 0:64], op=OP.add)
                nc.vector.tensor_tensor(out=ain[:, g], in0=oa[:, 64:96],
                                        in1=posoa[:, g, 64:96], op=OP.add)

            # attention softmax over NP (logits are small; skip max-sub)
            aw_e = achain.tile([P, G, 32], F32, tag="aw_e")
            nc.scalar.activation(out=aw_e[:].rearrange("p g a -> p (g a)"),
                                 in_=ain[:].rearrange("p g a -> p (g a)"), func=AF.Exp)
            aw_s = achain.tile([P, G, 8], F32, tag="aw_s")
            nc.vector.reduce_sum(out=aw_s[:],
                                 in_=aw_e[:].rearrange("p g (h q) -> p (g h) q", h=8),
                                 axis=mybir.AxisListType.X)
            nc.vector.reciprocal(out=aw_s[:].rearrange("p g h -> p (g h)"),
                                 in_=aw_s[:].rearrange("p g h -> p (g h)"))
            aw = achain.tile([P, G, 32], F32, tag="aw")
            nc.vector.tensor_tensor(
                out=aw[:].rearrange("p g (h q) -> p g h q", h=8),
                in0=aw_e[:].rearrange("p g (h q) -> p g h q", h=8),
                in1=_ap(aw_s[:], 0, [aw_s[:].ap[0], [8, G], [1, 8], [0, NP]]),
                op=OP.mult)

            # pixel coords: clip, floor via round(x+1023.5)-1024, fraction
            F = G * 64
            fl = lambda x: x[:].rearrange("p g a -> p (g a)")
            pc = achain.tile([P, G, 64], F32, tag="pc")
            nc.vector.tensor_scalar(out=fl(pc), in0=fl(px), scalar1=-2.0,
                                    scalar2=66.0, op0=OP.max, op1=OP.min)
            t2 = achain.tile([P, G, 64], F32, tag="t2")
            nc.vector.tensor_scalar_add(fl(t2), fl(pc), 1023.5)
            pi = achain.tile([P, G, 64], I32, tag="px")
            nc.vector.tensor_copy(out=fl(pi), in_=fl(t2))
            pf = achain.tile([P, G, 64], F32, tag="t2")
            nc.vector.tensor_copy(out=fl(pf), in_=fl(pi))
            x0 = achain.tile([P, G, 64], F32, tag="x0")
            nc.vector.tensor_scalar_add(fl(x0), fl(pf), -1024.0)
            wf = achain.tile([P, G, 64], F32, tag="wf")
            nc.vector.scalar_tensor_tensor(out=fl(wf), in0=fl(pc), scalar=1024.0,
                                           in1=fl(pf), op0=OP.add, op1=OP.subtract)
            xs = achain.tile([P, G, 64], F32, tag="xs")
            nc.vector.tensor_scalar(out=fl(xs), in0=fl(x0), scalar1=0.0,
                                    scalar2=63.0, op0=OP.max, op1=OP.min)
            eq0 = achain.tile([P, G, 64], F32, tag="eq0")
            nc.vector.tensor_tensor(out=fl(eq0), in0=fl(xs), in1=fl(x0),
                                    op=OP.is_equal)
            eqm1 = achain.tile([P, G, 64], F32, tag="eqm1")
            nc.vector.tensor_scalar(out=fl(eqm1), in0=fl(x0), scalar1=-1.0,
                                    scalar2=None, op0=OP.is_equal)
            eq2 = achain.tile([P, G, 64], F32, tag="eq2")
            nc.vector.tensor_scalar(out=fl(eq2), in0=fl(xs), scalar1=62.0,
                                    scalar2=None, op0=OP.min)
            nc.vector.tensor_tensor(out=fl(eq2), in0=fl(eq2), in1=fl(x0),
                                    op=OP.is_equal)
            # lane weights: CL = (1-wf)*eq0 + wf*eqm1 ; CR = wf*eq2
            u = achain.tile([P, G, 64], F32, tag="u")
            nc.vector.tensor_scalar(out=fl(u), in0=fl(wf), scalar1=1.0,
                                    scalar2=-1.0, op0=OP.subtract, op1=OP.mult)
            cl = achain.tile([P, G, 64], F32, tag="cl")
            nc.vector.tensor_tensor(out=fl(cl), in0=fl(u), in1=fl(eq0), op=OP.mult)
            tmp = achain.tile([P, G, 64], F32, tag="tmp")
            nc.vector.tensor_tensor(out=fl(tmp), in0=fl(wf), in1=fl(eqm1), op=OP.mult)
            nc.vector.tensor_tensor(out=fl(cl), in0=fl(cl), in1=fl(tmp), op=OP.add)
            cr = achain.tile([P, G, 64], F32, tag="cr")
            nc.vector.tensor_tensor(out=fl(cr), in0=fl(wf), in1=fl(eq2), op=OP.mult)

            # fold attention weight into the y-lane weights
            def lane(tl, c):
                return _ap(tl[:], c, [tl[:].ap[0], [64, G], [2, 32]])
            nc.vector.tensor_tensor(out=lane(cl, 1), in0=lane(cl, 1),
                                    in1=aw[:], op=OP.mult)
            nc.vector.tensor_tensor(out=lane(cr, 1), in0=lane(cr, 1),
                                    in1=aw[:], op=OP.mult)

            # W4[q, t, (h,p), (r,c)] = Y_r * X_c
            for r, yt in ((0, cl), (1, cr)):
                for cc, xt in ((0, cl), (1, cr)):
                    nc.vector.tensor_tensor(
                        out=_ap(w4_all[:], t0g * P + r * 2 + cc,
                                [w4_all[:].ap[0], [P, G], [4, 32]]),
                        in0=lane(yt, 1), in1=lane(xt, 0), op=OP.mult)

            # gather row index: (ys*64 + xs)*8 + h
            idxf = achain.tile([P, G, 32], F32, tag="idxf")
            nc.vector.scalar_tensor_tensor(
                out=idxf[:], in0=lane(xs, 1), scalar=512.0,
                in1=_ap(hcst[:], 0, [hcst[:].ap[0], [0, G], [1, 32]]),
                op0=OP.mult, op1=OP.add)
            xs8 = achain.tile([P, G, 32], F32, tag="xs8")
            nc.vector.tensor_scalar(out=xs8[:], in0=lane(xs, 0), scalar1=8.0,
                                    scalar2=None, op0=OP.mult)
            nc.vector.tensor_tensor(out=idxf[:].rearrange("p g a -> p (g a)"),
                                    in0=idxf[:].rearrange("p g a -> p (g a)"),
                                    in1=xs8[:].rearrange("p g a -> p (g a)"),
                                    op=OP.add)

            for g in range(G):
                t = t0g + g
                # -> wrapped int16 layout via PE transpose + shuffled DRAM write
                tpi = ppt.tile([P, P], F32, tag="pt")
                nc.tensor.transpose(tpi[0:32, :], idxf[:, g], ident[:])
                idx16 = work.tile([32, P], I16, tag="idx16")
                nc.vector.tensor_copy(
                    out=idx16[:].rearrange("p (a c) -> p a c", a=16),
                    in_=tpi[0:32, :].rearrange("p (a c) -> p c a", a=8))
                nc.sync.dma_start(
                    out=bass.AP(tensor=idxshuf_ds[b][:].tensor,
                                offset=t * 4096,
                                ap=[[8, 32], [256, 16], [1, 8]]),
                    in_=idx16[:].rearrange("p (a c) -> p a c", a=16))

                # 4x shifted value projection -> patch rows [q, h, d, c]
                h2sb = io.tile([P, NH, DH, 4], BF16, tag="h2sb")
                for ci, dlt in enumerate((0, 1, WS, WS + 1)):
                    vp = ppv.tile([P, C], F32, tag="pv")
                    nc.tensor.matmul(vp[:], srcT[:, 0, t * P + dlt:t * P + dlt + P],
                                     wval[:, 0, :], start=True, stop=False)
                    nc.tensor.matmul(vp[:], srcT[:, 1, t * P + dlt:t * P + dlt + P],
                                     wval[:, 1, :], start=False, stop=True)
                    if nz.get("b_val"):
                        nc.vector.tensor_tensor(
                            out=h2sb[:, :, :, ci],
                            in0=vp[:].rearrange("p (h d) -> p h d", h=NH),
                            in1=cext["b_val"][:].rearrange("p (h d) -> p h d", h=NH),
                            op=OP.add)
                    else:
                        nc.scalar.copy(
                            out=h2sb[:, :, :, ci],
                            in_=vp[:].rearrange("p (h d) -> p h d", h=NH))
                nc.sync.dma_start(
                    out=h2_ds[b][t * P * NH:(t + 1) * P * NH, :],
                    in_=h2sb[:].rearrange("p h d c -> p (h d c)"))

        def emit_idxload(b):
            # gather indices for this batch into SBUF (8 Q7 core replicas)
            for g in range(8):
                nc.sync.dma_start(
                    out=idx_alls[b][16 * g:16 * (g + 1), :, :],
                    in_=bass.AP(tensor=idxshuf_ds[b][:].tensor, offset=0,
                                ap=[[256, 16], [4096, NT], [1, 256]]))

        def emit_B(b, t):
            """Gather, weighted corner sum, out-proj, LN1, FFN, LN2, store."""
            srcT, idx_all, w4_all = srcTs[b], idx_alls[b], w4_alls[b]
            h2b = h2_ds[b][:, :]
            r0 = b * LQ + t * P
            gat = patches_p.tile([P, 32, 4 * DH], BF16, tag="gat")
            for gq in range(4):
                nc.gpsimd.dma_gather(
                    gat[:, gq * 8:(gq + 1) * 8, :], h2b,
                    idx_all[:, t, gq * 64:(gq + 1) * 64],
                    1024, 1024, 4 * DH)

            # weighted 16-term corner/point sum, all in 2x-mode bf16:
            # mac[q, (h,p), d, c] = gat * w4 (w4 broadcast over d), then
            # halving adds: p-pairs, p-final, c-pairs, c-final.
            mac = work.tile([P, 4096], BF16, tag="mac")
            w4t = w4_all[:, t]
            w4v = _ap(w4t, 0, [w4t.ap[0], [4, 32], [0, DH], [1, 4]])
            nc.vector.tensor_tensor(
                out=mac[:].rearrange("p (j d c) -> p j d c", j=32, c=4),
                in0=gat[:].rearrange("p j (d c) -> p j d c", c=4),
                in1=w4v, op=OP.mult)
            r1 = work.tile([P, 2048], BF16, tag="r1")
            nc.vector.tensor_tensor(
                out=r1[:],
                in0=_ap(mac[:], 0, [mac[:].ap[0], [512, 8], [1, 256]]),
                in1=_ap(mac[:], 256, [mac[:].ap[0], [512, 8], [1, 256]]),
                op=OP.add)
            r2 = work.tile([P, 1024], BF16, tag="r2")
            nc.vector.tensor_tensor(
                out=r2[:],
                in0=_ap(r1[:], 0, [r1[:].ap[0], [256, 8], [1, 128]]),
                in1=_ap(r1[:], 128, [r1[:].ap[0], [256, 8], [1, 128]]),
                op=OP.add)
            r3 = work.tile([P, 512], BF16, tag="r3")
            nc.vector.tensor_tensor(
                out=r3[:],
                in0=_ap(r2[:], 0, [r2[:].ap[0], [4, 256], [1, 2]]),
                in1=_ap(r2[:], 2, [r2[:].ap[0], [4, 256], [1, 2]]),
                op=OP.add)
            gt = work.tile([P, C], F32, tag="gt")
            nc.vector.tensor_tensor(
                out=gt[:],
                in0=_ap(r3[:], 0, [r3[:].ap[0], [2, 256]]),
                in1=_ap(r3[:], 1, [r3[:].ap[0], [2, 256]]),
                op=OP.add)

            # out projection needs G^T
            gT = work.tile([P, 2, P], BF16, tag="gT")
            for k in range(2):
                tp = ppt.tile([P, P], F32, tag="pt")
                nc.tensor.transpose(tp[:], gt[:, 128 * k:128 * (k + 1)], ident[:])
                nc.scalar.copy(out=gT[:, k, :], in_=tp[:])

            # s1 = src + attn_out: src chunks land via idstack matmuls
            s1 = pps.tile([P, C + 1], F32, tag="ps")
            for k in range(2):
                nc.tensor.matmul(s1[:], srcT[:, k, t * P:(t + 1) * P],
                                 idstack[:, k, :], start=(k == 0), stop=False,
                                 skip_group_check=True)
            nc.tensor.matmul(s1[:], gT[:, 0, :], wout[:, 0, :], start=False,
                             stop=False, skip_group_check=True)
            nc.tensor.matmul(s1[:], gT[:, 1, :], wout[:, 1, :], start=False,
                             stop=True, skip_group_check=True)
            if nz.get("b_out"):
                nc.vector.tensor_tensor(out=s1[:, 0:C], in0=s1[:, 0:C],
                                        in1=cext["b_out"][:], op=OP.add)

            # x1 = LN1(s1)
            x1 = work.tile([P, C], F32, tag="x1")
            _layernorm(nc, work, x1, s1, cext.get("ln1_g"), cext.get("ln1_b"), epsb)

            # FFN
            x1T = work.tile([P, 2, P], BF16, tag="x1T")
            for k in range(2):
                tp = ppt.tile([P, P], F32, tag="pt")
                nc.tensor.transpose(tp[:], x1[:, 128 * k:128 * (k + 1)], ident[:])
                nc.scalar.copy(out=x1T[:, k, :], in_=tp[:])
            hT = work.tile([P, 8, P], BF16, tag="hT")
            for j in range(8):
                fp = ppf.tile([P, P], F32, tag="pf")
                nc.tensor.matmul(fp[:], w1[:, 0, 128 * j:128 * (j + 1)], x1T[:, 0, :],
                                 start=True, stop=False)
                nc.tensor.matmul(fp[:], w1[:, 1, 128 * j:128 * (j + 1)], x1T[:, 1, :],
                                 start=False, stop=True)
                nc.scalar.activation(out=hT[:, j, :], in_=fp[:], func=AF.Relu,
                                     bias=b1t[:, j:j + 1])
            # s2 = x1 + ffn: FFN2 matmuls then x1T idstack-accumulate
            s2 = pps.tile([P, C + 1], F32, tag="ps")
            for j in range(8):
                nc.tensor.matmul(s2[:], hT[:, j, :], w2[:, j, :],
                                 start=(j == 0), stop=False,
                                 skip_group_check=True)
            for k in range(2):
                nc.tensor.matmul(s2[:], x1T[:, k, :], idstack[:, k, :],
                                 start=False, stop=(k == 1),
                                 skip_group_check=True)
            if nz.get("b2"):
                nc.vector.tensor_tensor(out=s2[:, 0:C], in0=s2[:, 0:C],
                                        in1=cext["b2"][:], op=OP.add)
            o_t = io.tile([P, C], BF16, tag="o_t")
            _layernorm(nc, work, o_t, s2, cext.get("ln2_g"), cext.get("ln2_b"), epsb)
            nc.sync.dma_start(out=out_d[r0:r0 + P, :], in_=o_t[:])

        # ---- software-pipelined emission: A(0) | B(0)+A(1) | B(1) ----
        setup_batch(0)
        for tg in range(NG):
            emit_A_group(0, tg)
        emit_idxload(0)
        setup_batch(1)
        for t in range(NT):
            emit_B(0, t)
            if t % 2 == 1 and t // 2 < NG:
                emit_A_group(1, t // 2)
            if t == 2 * NG + 1:
                emit_idxload(1)
        for t in range(NT):
            emit_B(1, t)

    nc.compile()
    return nc


def _layernorm(nc, work, out_t, s, g_rep, b_rep, epsb):
    """s: PSUM [P, 257]; col 256 = mean(s[:, 0:256]) from the matmul mean
    column. Variance = E[x^2] - mean^2 (means here are ~0: no cancellation);
    E[x^2] accumulates on the Activation engine via Square+accum_out."""
    x = s[:, 0:C]
    mean = s[:, C:C + 1]
    sqo = work.tile([P, C], F32, tag="ln_sq")
    ssq = work.tile([P, 1], F32, tag="ln_ssq")
    nc.scalar.activation(out=sqo[:], in_=x, func=AF.Square, accum_out=ssq[:])
    m2 = work.tile([P, 1], F32, tag="ln_m2")
    nc.scalar.activation(out=m2[:], in_=mean, func=AF.Square)
    var = work.tile([P, 1], F32, tag="ln_var")
    nc.vector.scalar_tensor_tensor(out=var[:], in0=ssq[:], scalar=1.0 / C,
                                   in1=m2[:], op0=OP.mult, op1=OP.subtract)
    rstd = work.tile([P, 1], F32, tag="ln_rstd")
    nc.vector.tensor_scalar(out=rstd[:], in0=var[:], scalar1=1e-5,
                            scalar2=None, op0=OP.add)
    nc.vector.reciprocal(out=rstd[:], in_=rstd[:])
    nc.scalar.activation(out=rstd[:], in_=rstd[:], func=AF.Sqrt)
    negmr = work.tile([P, 1], F32, tag="ln_negmr")
    nc.vector.scalar_tensor_tensor(out=negmr[:], in0=mean, scalar=-1.0,
                                   in1=rstd[:], op0=OP.mult, op1=OP.mult)
    if g_rep is None and b_rep is None:
        nc.scalar.activation(out=out_t[:], in_=x, func=AF.Identity,
                             bias=negmr[:, 0:1], scale=rstd[:, 0:1])
        return
    xn = work.tile([P, C], F32, tag="ln_xn")
    nc.scalar.activation(out=xn[:], in_=x, func=AF.Identity,
                         bias=negmr[:, 0:1], scale=rstd[:, 0:1])
    if g_rep is not None and b_rep is not None:
        nc.vector.tensor_tensor(out=xn[:], in0=xn[:], in1=g_rep[:], op=OP.mult)
        nc.vector.tensor_tensor(out=out_t[:], in0=xn[:], in1=b_rep[:], op=OP.add)
    elif g_rep is not None:
        nc.vector.tensor_tensor(out=out_t[:], in0=xn[:], in1=g_rep[:], op=OP.mult)
    else:
        nc.vector.tensor_tensor(out=out_t[:], in0=xn[:], in1=b_rep[:], op=OP.add)


def make_in_maps(inputs):
    import ml_dtypes
    src = np.ascontiguousarray(np.asarray(inputs["src"], dtype=np.float32))
    pos = np.asarray(inputs["pos"], dtype=np.float32)
    ref = np.asarray(inputs["reference_points"], dtype=np.float32)[:, :, 0, :]
    w = {}
    for n in ("W1", "W_off", "W_attn", "W_val"):
        w[n] = np.ascontiguousarray(
            np.asarray(inputs[n], dtype=np.float32).astype(ml_dtypes.bfloat16))
    # W_out / W2 gain a 257th column = row-mean/... the LN mean columns
    for n in ("W_out", "W2"):
        wf = np.asarray(inputs[n], dtype=np.float32)
        wx = np.concatenate([wf, wf.mean(axis=1, keepdims=True)], axis=1)
        w[n] = np.ascontiguousarray(wx.astype(ml_dtypes.bfloat16))
    w["b1"] = np.ascontiguousarray(np.asarray(inputs["b1"], dtype=np.float32))

    # pos@[W_off|W_attn] (+ biases), with the pixel-space reference point
    # folded into the offset lanes: px = src@W_off + posoa = loc*64 - 0.5
    w_off = np.asarray(inputs["W_off"], dtype=np.float32)
    w_attn = np.asarray(inputs["W_attn"], dtype=np.float32)
    posoa = np.empty((B, LQ, 96), dtype=np.float32)
    pos2 = pos.reshape(B * LQ, C)
    posoa[:, :, 0:64] = (pos2 @ w_off + np.asarray(inputs["b_off"], np.float32)
                         ).reshape(B, LQ, 64)
    posoa[:, :, 0:64] += np.tile(ref * 64.0 - 0.5, (1, 1, 32))
    posoa[:, :, 64:96] = (pos2 @ w_attn + np.asarray(inputs["b_attn"], np.float32)
                          ).reshape(B, LQ, 32)

    in_maps = []
    for c in range(NCORES):
        m = dict(w)
        sc = src[BPC * c:BPC * (c + 1)]
        m["srcT"] = np.ascontiguousarray(
            sc.transpose(0, 2, 1).astype(ml_dtypes.bfloat16)).reshape(BPC * C, LQ)
        # regroup to [b, tile-group, q-within-tile, tile-in-group, 96] so
        # each SBUF partition's group slice is one contiguous DMA chunk
        pg = posoa[BPC * c:BPC * (c + 1)].reshape(BPC, LQ // (4 * P), 4, P, 96)
        m["posoa"] = np.ascontiguousarray(
            pg.transpose(0, 1, 3, 2, 4)).reshape(BPC * LQ, 96)
        in_maps.append(m)
    return in_maps


def assemble_output(results):
    out = np.stack([np.asarray(results[c]["out"], dtype=np.float32
                               ).reshape(BPC, LQ, C) for c in range(NCORES)])
    return out.reshape(B, LQ, C)


def kernel(**inputs):
    if "nc" not in _CACHE:
        _CACHE["nc"] = build_nc({k: inputs[k] for k in
                                 ("b_val", "b_out", "b2",
                                  "ln1_g", "ln1_b", "ln2_g", "ln2_b")})
    nc = _CACHE["nc"]
    in_maps = make_in_maps(inputs)
    res = run_bass_kernel_spmd(nc, in_maps, core_ids=list(range(NCORES)))
    return assemble_output(res.results)


# revision 69
# speedup vs baseline: 3.7737x; 3.7737x over previous
"""Deformable-attention encoder layer (single level) on 8 Trainium2 cores.

Data-parallel over batch: B=16 -> 2 images per NeuronCore. Each core runs the
full layer for its 2 images; no collectives.

MSDA sampling strategy: the value projection is computed 4 times with the src
rows shifted by (0, 1, 64, 65), giving, for every spatial cell e=(y,x), the
full 2x2 bilinear patch per head laid out [d(32), c(4)] (128 bf16 = 256B).
Patches go to a DRAM table indexed by (e, head) and are fetched with gpsimd
dma_gather in chunks of 1024 indices (>=2048 indices per gather overflows the
SWDGE descriptor ring on this runtime and wedges the device). The int16 index
tile must be in dma_gather's wrapped layout (index i at partition i%16, column
i//16, replicated across the 8 Q7 core groups), produced by a PE transpose +
a shuffled DRAM round trip.

v2 changes vs the 1.30ms baseline (cost-model-guided; DVE was 78% busy at
1.37ms simulated, this version simulates at 0.79ms with DVE/DMA balanced
at ~62% each):
- pos@[W_off|W_attn] + ref*64-0.5 + b_off|b_attn precomputed on host into
  a 96-float "posoa" stream: kills the posT loads, 2 of 4 q-proj matmuls
  and the on-device reference-point math at full fp32 precision.
- residual adds run on the PE: src / x1 chunks enter the out-proj / FFN2
  PSUM accumulation via [0|I|0] "idstack" matmuls from the resident
  srcT/x1T tiles; the fp32 src stream and two DVE adds are gone.
- the 16-term bilinear corner sum runs as 4 strided halving tensor_tensor
  adds (mostly 2x DVE mode) instead of a 1x TensorReduce + 3 tree adds.
- phase-A index math is batched over G=4 tiles per instruction (amortizes
  the 58-cycle DVE instruction overhead).
- PSUM->SBUF moves (patch rows, gT, x1T) run on the Activation engine.
- zero biases / unit LN gains (per the generating spec) skip their ops;
  build_nc checks the actual input values and only emits the adds when
  nonzero (from inline constants), so generality is preserved.
- floor() via int32 round trip of x+1023.5 (round-to-nearest; a flip at
  exact integers lands the full weight on the equivalent other corner).
  NOTE: the fp32 +1.5*2^23 magic-number floor is numerically broken on
  real DVE hardware (0.26 rel err) despite being architecturally sound -
  keep the int cast.
- output stream is bf16 (host upcasts): halves the final store DMA for
  ~2e-3 extra mean-relative error against a 2e-2 budget.
- emission is software-pipelined A(0) | B(0)+A(1) | B(1) with per-batch
  DRAM tables: batch 1's projections/table writes overlap batch 0's
  gather+attention phase. Engine SEQ queues are in-order, so overlap
  exists only if instruction emission interleaves.

Tried and rejected (cost-model evidence): 2048-idx gathers w/ enlarged
SWDGE ring (Pool halves, longer exclusive DMA holds cost more); G=8
batching; table-first prologue and per-group idx/w4 tiles (extra DMAs +
dep edges cost ~30us; B(0) is gated on the full table either way); deeper
work/mac pools; ppt/ppv/ppf rebalancing.
"""

import sys
from contextlib import ExitStack

import numpy as np

sys.path.insert(0, "/opt/trn_rl_repo")

import concourse.bass as bass
import concourse.bacc as bacc
import concourse.mybir as mybir
import concourse.tile as tile
from concourse.bass_utils import run_bass_kernel_spmd

F32 = mybir.dt.float32
BF16 = mybir.dt.bfloat16
I16 = mybir.dt.int16
I32 = mybir.dt.int32
AF = mybir.ActivationFunctionType
OP = mybir.AluOpType

B, LQ, C = 16, 4096, 256
NH, NP, DH = 8, 4, 32
HS, WS = 64, 64
DFF = 1024
NCORES = 8
BPC = B // NCORES          # batches per core
P = 128
NT = LQ // P               # q-tiles per batch
EPAD = 128                 # srcT column padding for shifted windows

_CACHE = {}


def _bcast_ap(handle_or_ap, n_free):
    """DRAM [n] -> AP [128, n] replicated across partitions."""
    ap = handle_or_ap[:] if not isinstance(handle_or_ap, bass.AP) else handle_or_ap
    return bass.AP(tensor=ap.tensor, offset=ap.offset, ap=[[0, P], [1, n_free]])


def _ap(t, offset, dims):
    return bass.AP(tensor=t.tensor, offset=t.offset + offset, ap=dims)


def build_nc(consts_in):
    """consts_in: dict with the (host-checked) bias / LN parameter values so
    ops for all-zero biases / unit gains are skipped."""
    nz = {k: bool(np.any(np.asarray(v) != 0)) for k, v in consts_in.items()
          if k in ("b_val", "b_out", "b2", "ln1_b", "ln2_b")}
    nz["ln1_g"] = bool(np.any(np.asarray(consts_in["ln1_g"]) != 1))
    nz["ln2_g"] = bool(np.any(np.asarray(consts_in["ln2_g"]) != 1))

    nc = bacc.Bacc("TRN2")

    # host-pretransposed [batch*C, LQ] copy feeding the matmul lhsT layout
    srcT_d = nc.dram_tensor("srcT", [BPC * C, LQ], BF16, kind="ExternalInput")
    # host-precomputed pos@[W_off|W_attn] + [ref*64-0.5 | 0] + [b_off|b_attn]
    posoa_d = nc.dram_tensor("posoa", [BPC * LQ, 96], F32, kind="ExternalInput")
    w_off_d = nc.dram_tensor("W_off", [C, 64], BF16, kind="ExternalInput")
    w_attn_d = nc.dram_tensor("W_attn", [C, 32], BF16, kind="ExternalInput")
    w_val_d = nc.dram_tensor("W_val", [C, C], BF16, kind="ExternalInput")
    w_out_d = nc.dram_tensor("W_out", [C, C + 1], BF16, kind="ExternalInput")
    w1_d = nc.dram_tensor("W1", [C, DFF], BF16, kind="ExternalInput")
    b1_d = nc.dram_tensor("b1", [DFF], F32, kind="ExternalInput")
    w2_d = nc.dram_tensor("W2", [DFF, C + 1], BF16, kind="ExternalInput")
    out_d = nc.dram_tensor("out", [BPC * LQ, C], BF16, kind="ExternalOutput")

    # patch table: row (e*8 + h), one tensor PER BATCH so phase A of batch
    # b+1 (table writes) doesn't serialize behind batch b's gathers
    h2_ds = [nc.dram_tensor(f"h2tab{b}", [LQ * NH, 4 * DH], BF16, kind="Internal")
             for b in range(BPC)]
    # per-tile gather indices in dma_gather's wrapped layout, int16
    idxshuf_ds = [nc.dram_tensor(f"idxshuf{b}", [NT * P * 32], I16, kind="Internal")
                  for b in range(BPC)]

    import ml_dtypes
    ident_d = nc.inline_tensor(np.eye(P, dtype=np.float32), "identconst")
    identb_d = nc.inline_tensor(
        np.eye(P, dtype=np.float32).astype(ml_dtypes.bfloat16), "identbconst")
    # idstack[k] = [0 .. I .. 0]: srcT_k.T @ idstack[k] drops the src chunk
    # into column block k of an accumulating PSUM tile (residual adds on PE)
    idstack_np = np.zeros((P, 2, C + 1), dtype=np.float32)
    idstack_np[:, 0, 0:P] = np.eye(P)
    idstack_np[:, 1, P:C] = np.eye(P)
    idstack_np[:, :, C] = 1.0 / C            # mean column (2^-8, bf16-exact)
    idstack_d = nc.inline_tensor(
        idstack_np.reshape(P, 2 * (C + 1)).astype(ml_dtypes.bfloat16),
        "idstackconst")
    # per-(h,p) head index constant, replicated over partitions
    hc = np.tile(np.repeat(np.arange(NH, dtype=np.float32), NP)[None, :], (P, 1))
    hc_d = nc.inline_tensor(hc, "hconst")

    def _maybe_const(name, vals, n):
        """[128, n]-replicated inline const for a nonzero bias vector."""
        arr = np.tile(np.asarray(vals, dtype=np.float32)[None, :], (P, 1))
        return nc.inline_tensor(arr, name)

    with ExitStack() as ctx:
        tc = ctx.enter_context(tile.TileContext(nc))
        consts = ctx.enter_context(tc.tile_pool(name="consts", bufs=1))
        persist = ctx.enter_context(tc.tile_pool(name="persist", bufs=1))
        work = ctx.enter_context(tc.tile_pool(name="work", bufs=2))
        io = ctx.enter_context(tc.tile_pool(name="io", bufs=4))
        achain = ctx.enter_context(tc.tile_pool(name="achain", bufs=1))
        patches_p = ctx.enter_context(tc.tile_pool(name="patches", bufs=2))
        ppt = ctx.enter_context(tc.tile_pool(name="ppt", bufs=2, space="PSUM"))
        pps = ctx.enter_context(tc.tile_pool(name="pps", bufs=2, space="PSUM"))
        ppv = ctx.enter_context(tc.tile_pool(name="ppv", bufs=2, space="PSUM"))
        ppf = ctx.enter_context(tc.tile_pool(name="ppf", bufs=2, space="PSUM"))

        # ---- constants / weights into SBUF ----
        ident = consts.tile([P, P], F32)
        nc.sync.dma_start(out=ident[:], in_=ident_d[:, :])
        identb = consts.tile([P, P], BF16)
        nc.sync.dma_start(out=identb[:], in_=identb_d[:, :])
        idstack = consts.tile([P, 2, C + 1], BF16)
        nc.sync.dma_start(out=idstack[:].rearrange("p k c -> p (k c)"),
                          in_=idstack_d[:, :])
        hcst = consts.tile([P, 32], F32)
        nc.sync.dma_start(out=hcst[:], in_=hc_d[:, :])
        epsb = consts.tile([P, 1], F32)
        nc.vector.memset(epsb[:], 1e-5)

        wb = consts.tile([P, 2, 96], BF16)          # W_off|W_attn, 2 k-tiles
        for k in range(2):
            nc.sync.dma_start(out=wb[:, k, 0:64], in_=w_off_d[128 * k:128 * (k + 1), :])
            nc.sync.dma_start(out=wb[:, k, 64:96], in_=w_attn_d[128 * k:128 * (k + 1), :])
        wval = consts.tile([P, 2, C], BF16)
        wout = consts.tile([P, 2, C + 1], BF16)
        for k in range(2):
            nc.sync.dma_start(out=wval[:, k, :], in_=w_val_d[128 * k:128 * (k + 1), :])
            nc.sync.dma_start(out=wout[:, k, :], in_=w_out_d[128 * k:128 * (k + 1), :])
        w1 = consts.tile([P, 2, DFF], BF16)
        for k in range(2):
            nc.sync.dma_start(out=w1[:, k, :], in_=w1_d[128 * k:128 * (k + 1), :])
        w2 = consts.tile([P, 8, C + 1], BF16)
        for j in range(8):
            nc.sync.dma_start(out=w2[:, j, :], in_=w2_d[128 * j:128 * (j + 1), :])

        # b1 transposed: [128, 8] with b1t[p, j] = b1[128j + p] (free via ACT bias)
        b1t = consts.tile([P, 8], F32)
        b1_ap = bass.AP(tensor=b1_d[:].tensor, offset=0, ap=[[1, P], [P, 8]])
        nc.sync.dma_start(out=b1t[:], in_=b1_ap)

        # optional nonzero bias / non-unit LN constants
        cext = {}
        for name, width in (("b_val", C), ("b_out", C), ("b2", C),
                            ("ln1_g", C), ("ln1_b", C),
                            ("ln2_g", C), ("ln2_b", C)):
            if nz.get(name):
                d = _maybe_const(name + "c", np.asarray(consts_in[name]), width)
                tl = consts.tile([P, width], F32)
                nc.sync.dma_start(out=tl[:], in_=d[:, :])
                cext[name] = tl

        G = 4                    # phase-A tiles batched per index-math group
        NG = NT // G
        srcTs, idx_alls, w4_alls = {}, {}, {}

        def setup_batch(b):
            srcTs[b] = persist.tile([P, 2, LQ + EPAD], BF16, tag=f"srcT{b}", name=f"srcT{b}")
            for k in range(2):
                nc.vector.memset(srcTs[b][:, k, LQ:LQ + EPAD], 0.0)
                nc.sync.dma_start(
                    out=srcTs[b][:, k, 0:LQ],
                    in_=srcT_d[b * 2 * P + k * P:b * 2 * P + (k + 1) * P, :])
            idx_alls[b] = persist.tile([P, NT, 256], I16, tag=f"idx_all{b}", name=f"idx_all{b}")
            w4_alls[b] = persist.tile([P, NT, P], BF16, tag=f"w4_all{b}", name=f"w4_all{b}")

        def emit_A_group(b, tg):
            """Index math / weights / patch table for tiles tg*G..tg*G+G-1."""
            srcT, w4_all = srcTs[b], w4_alls[b]
            t0g = tg * G
            r0g = b * LQ + t0g * P
            # host pre-grouped layout [b, tg, p, g, 96]: one contiguous
            # 1536B chunk per partition (no small-transfer DMA penalty)
            posoa = io.tile([P, G, 96], F32, tag="posoa")
            nc.sync.dma_start(
                out=posoa[:],
                in_=_ap(posoa_d[:], (b * NG + tg) * P * G * 96,
                        [[G * 96, P], [1, G * 96]]))

            # oa projections per tile; px/ain collected into group tiles
            px = achain.tile([P, G, 64], F32, tag="px")
            ain = achain.tile([P, G, 32], F32, tag="ain")
            for g in range(G):
                t = t0g + g
                oa = ppv.tile([P, 96], F32, tag="pv")
                nc.tensor.matmul(oa[:, 0:96], srcT[:, 0, t * P:(t + 1) * P],
                                 wb[:, 0, :], start=True, stop=False)
                nc.tensor.matmul(oa[:, 0:96], srcT[:, 1, t * P:(t + 1) * P],
                                 wb[:, 1, :], start=False, stop=True)
                nc.vector.tensor_tensor(out=px[:, g], in0=oa[:, 0:64],
                                        in1=posoa[:, g, 0:64], op=OP.add)
                nc.vector.tensor_tensor(out=ain[:, g], in0=oa[:, 64:96],
                                        in1=posoa[:, g, 64:96], op=OP.add)

            # attention softmax over NP (logits are small; skip max-sub)
            aw_e = achain.tile([P, G, 32], F32, tag="aw_e")
            nc.scalar.activation(out=aw_e[:].rearrange("p g a -> p (g a)"),
                                 in_=ain[:].rearrange("p g a -> p (g a)"), func=AF.Exp)
            aw_s = achain.tile([P, G, 8], F32, tag="aw_s")
            nc.vector.reduce_sum(out=aw_s[:],
                                 in_=aw_e[:].rearrange("p g (h q) -> p (g h) q", h=8),
                                 axis=mybir.AxisListType.X)
            nc.vector.reciprocal(out=aw_s[:].rearrange("p g h -> p (g h)"),
                                 in_=aw_s[:].rearrange("p g h -> p (g h)"))
            aw = achain.tile([P, G, 32], F32, tag="aw")
            nc.vector.tensor_tensor(
                out=aw[:].rearrange("p g (h q) -> p g h q", h=8),
                in0=aw_e[:].rearrange("p g (h q) -> p g h q", h=8),
                in1=_ap(aw_s[:], 0, [aw_s[:].ap[0], [8, G], [1, 8], [0, NP]]),
                op=OP.mult)

            # pixel coords: clip, floor via round(x+1023.5)-1024, fraction
            F = G * 64
            fl = lambda x: x[:].rearrange("p g a -> p (g a)")
            pc = achain.tile([P, G, 64], F32, tag="pc")
            nc.vector.tensor_scalar(out=fl(pc), in0=fl(px), scalar1=-2.0,
                                    scalar2=66.0, op0=OP.max, op1=OP.min)
            t2 = achain.tile([P, G, 64], F32, tag="t2")
            nc.vector.tensor_scalar_add(fl(t2), fl(pc), 1023.5)
            pi = achain.tile([P, G, 64], I32, tag="px")
            nc.vector.tensor_copy(out=fl(pi), in_=fl(t2))
            pf = achain.tile([P, G, 64], F32, tag="t2")
            nc.vector.tensor_copy(out=fl(pf), in_=fl(pi))
            x0 = achain.tile([P, G, 64], F32, tag="x0")
            nc.vector.tensor_scalar_add(fl(x0), fl(pf), -1024.0)
            wf = achain.tile([P, G, 64], F32, tag="wf")
            nc.vector.scalar_tensor_tensor(out=fl(wf), in0=fl(pc), scalar=1024.0,
                                           in1=fl(pf), op0=OP.add, op1=OP.subtract)
            xs = achain.tile([P, G, 64], F32, tag="xs")
            nc.vector.tensor_scalar(out=fl(xs), in0=fl(x0), scalar1=0.0,
                                    scalar2=63.0, op0=OP.max, op1=OP.min)
            eq0 = achain.tile([P, G, 64], F32, tag="eq0")
            nc.vector.tensor_tensor(out=fl(eq0), in0=fl(xs), in1=fl(x0),
                                    op=OP.is_equal)
            eqm1 = achain.tile([P, G, 64], F32, tag="eqm1")
            nc.vector.tensor_scalar(out=fl(eqm1), in0=fl(x0), scalar1=-1.0,
                                    scalar2=None, op0=OP.is_equal)
            eq2 = achain.tile([P, G, 64], F32, tag="eq2")
            nc.vector.tensor_scalar(out=fl(eq2), in0=fl(xs), scalar1=62.0,
                                    scalar2=None, op0=OP.min)
            nc.vector.tensor_tensor(out=fl(eq2), in0=fl(eq2), in1=fl(x0),
                                    op=OP.is_equal)
            # lane weights: CL = (1-wf)*eq0 + wf*eqm1 ; CR = wf*eq2
            u = achain.tile([P, G, 64], F32, tag="u")
            nc.vector.tensor_scalar(out=fl(u), in0=fl(wf), scalar1=1.0,
                                    scalar2=-1.0, op0=OP.subtract, op1=OP.mult)
            cl = achain.tile([P, G, 64], F32, tag="cl")
            nc.vector.tensor_tensor(out=fl(cl), in0=fl(u), in1=fl(eq0), op=OP.mult)
            tmp = achain.tile([P, G, 64], F32, tag="tmp")
            nc.vector.tensor_tensor(out=fl(tmp), in0=fl(wf), in1=fl(eqm1), op=OP.mult)
            nc.vector.tensor_tensor(out=fl(cl), in0=fl(cl), in1=fl(tmp), op=OP.add)
            cr = achain.tile([P, G, 64], F32, tag="cr")
            nc.vector.tensor_tensor(out=fl(cr), in0=fl(wf), in1=fl(eq2), op=OP.mult)

            # fold attention weight into the y-lane weights
            def lane(tl, c):
                return _ap(tl[:], c, [tl[:].ap[0], [64, G], [2, 32]])
            nc.vector.tensor_tensor(out=lane(cl, 1), in0=lane(cl, 1),
                                    in1=aw[:], op=OP.mult)
            nc.vector.tensor_tensor(out=lane(cr, 1), in0=lane(cr, 1),
                                    in1=aw[:], op=OP.mult)

            # W4[q, t, (h,p), (r,c)] = Y_r * X_c
            for r, yt in ((0, cl), (1, cr)):
                for cc, xt in ((0, cl), (1, cr)):
                    nc.vector.tensor_tensor(
                        out=_ap(w4_all[:], t0g * P + r * 2 + cc,
                                [w4_all[:].ap[0], [P, G], [4, 32]]),
                        in0=lane(yt, 1), in1=lane(xt, 0), op=OP.mult)

            # gather row index: (ys*64 + xs)*8 + h
            idxf = achain.tile([P, G, 32], F32, tag="idxf")
            nc.vector.scalar_tensor_tensor(
                out=idxf[:], in0=lane(xs, 1), scalar=512.0,
                in1=_ap(hcst[:], 0, [hcst[:].ap[0], [0, G], [1, 32]]),
                op0=OP.mult, op1=OP.add)
            xs8 = achain.tile([P, G, 32], F32, tag="xs8")
            nc.vector.tensor_scalar(out=xs8[:], in0=lane(xs, 0), scalar1=8.0,
                                    scalar2=None, op0=OP.mult)
            nc.vector.tensor_tensor(out=idxf[:].rearrange("p g a -> p (g a)"),
                                    in0=idxf[:].rearrange("p g a -> p (g a)"),
                                    in1=xs8[:].rearrange("p g a -> p (g a)"),
                                    op=OP.add)

            for g in range(G):
                t = t0g + g
                # -> wrapped int16 layout via PE transpose + shuffled DRAM write
                tpi = ppt.tile([P, P], F32, tag="pt")
                nc.tensor.transpose(tpi[0:32, :], idxf[:, g], ident[:])
                idx16 = work.tile([32, P], I16, tag="idx16")
                nc.vector.tensor_copy(
                    out=idx16[:].rearrange("p (a c) -> p a c", a=16),
                    in_=tpi[0:32, :].rearrange("p (a c) -> p c a", a=8))
                nc.sync.dma_start(
                    out=bass.AP(tensor=idxshuf_ds[b][:].tensor,
                                offset=t * 4096,
                                ap=[[8, 32], [256, 16], [1, 8]]),
                    in_=idx16[:].rearrange("p (a c) -> p a c", a=16))

                # 4x shifted value projection -> patch rows [q, h, d, c]
                h2sb = io.tile([P, NH, DH, 4], BF16, tag="h2sb")
                for ci, dlt in enumerate((0, 1, WS, WS + 1)):
                    vp = ppv.tile([P, C], F32, tag="pv")
                    nc.tensor.matmul(vp[:], srcT[:, 0, t * P + dlt:t * P + dlt + P],
                                     wval[:, 0, :], start=True, stop=False)
                    nc.tensor.matmul(vp[:], srcT[:, 1, t * P + dlt:t * P + dlt + P],
                                     wval[:, 1, :], start=False, stop=True)
                    if nz.get("b_val"):
                        nc.vector.tensor_tensor(
                            out=h2sb[:, :, :, ci],
                            in0=vp[:].rearrange("p (h d) -> p h d", h=NH),
                            in1=cext["b_val"][:].rearrange("p (h d) -> p h d", h=NH),
                            op=OP.add)
                    else:
                        nc.scalar.copy(
                            out=h2sb[:, :, :, ci],
                            in_=vp[:].rearrange("p (h d) -> p h d", h=NH))
                nc.sync.dma_start(
                    out=h2_ds[b][t * P * NH:(t + 1) * P * NH, :],
                    in_=h2sb[:].rearrange("p h d c -> p (h d c)"))

        def emit_idxload(b):
            # gather indices for this batch into SBUF (8 Q7 core replicas)
            for g in range(8):
                nc.sync.dma_start(
                    out=idx_alls[b][16 * g:16 * (g + 1), :, :],
                    in_=bass.AP(tensor=idxshuf_ds[b][:].tensor, offset=0,
                                ap=[[256, 16], [4096, NT], [1, 256]]))

        def emit_B(b, t):
            """Gather, weighted corner sum, out-proj, LN1, FFN, LN2, store."""
            srcT, idx_all, w4_all = srcTs[b], idx_alls[b], w4_alls[b]
            h2b = h2_ds[b][:, :]
            r0 = b * LQ + t * P
            gat = patches_p.tile([P, 32, 4 * DH], BF16, tag="gat")
            for gq in range(4):
                nc.gpsimd.dma_gather(
                    gat[:, gq * 8:(gq + 1) * 8, :], h2b,
                    idx_all[:, t, gq * 64:(gq + 1) * 64],
                    1024, 1024, 4 * DH)

            # weighted 16-term corner/point sum, all in 2x-mode bf16:
            # mac[q, (h,p), d, c] = gat * w4 (w4 broadcast over d), then
            # halving adds: p-pairs, p-final, c-pairs, c-final.
            mac = work.tile([P, 4096], BF16, tag="mac")
            w4t = w4_all[:, t]
            w4v = _ap(w4t, 0, [w4t.ap[0], [4, 32], [0, DH], [1, 4]])
            nc.vector.tensor_tensor(
                out=mac[:].rearrange("p (j d c) -> p j d c", j=32, c=4),
                in0=gat[:].rearrange("p j (d c) -> p j d c", c=4),
                in1=w4v, op=OP.mult)
            r1 = work.tile([P, 2048], BF16, tag="r1")
            nc.vector.tensor_tensor(
                out=r1[:],
                in0=_ap(mac[:], 0, [mac[:].ap[0], [512, 8], [1, 256]]),
                in1=_ap(mac[:], 256, [mac[:].ap[0], [512, 8], [1, 256]]),
                op=OP.add)
            r2 = work.tile([P, 1024], BF16, tag="r2")
            nc.vector.tensor_tensor(
                out=r2[:],
                in0=_ap(r1[:], 0, [r1[:].ap[0], [256, 8], [1, 128]]),
                in1=_ap(r1[:], 128, [r1[:].ap[0], [256, 8], [1, 128]]),
                op=OP.add)
            r3 = work.tile([P, 512], BF16, tag="r3")
            nc.vector.tensor_tensor(
                out=r3[:],
                in0=_ap(r2[:], 0, [r2[:].ap[0], [4, 256], [1, 2]]),
                in1=_ap(r2[:], 2, [r2[:].ap[0], [4, 256], [1, 2]]),
                op=OP.add)
            gt = work.tile([P, C], F32, tag="gt")
            nc.vector.tensor_tensor(
                out=gt[:],
                in0=_ap(r3[:], 0, [r3[:].ap[0], [2, 256]]),
                in1=_ap(r3[:], 1, [r3[:].ap[0], [2, 256]]),
                op=OP.add)

            # out projection needs G^T
            gT = work.tile([P, 2, P], BF16, tag="gT")
            for k in range(2):
                tp = ppt.tile([P, P], F32, tag="pt")
                nc.tensor.transpose(tp[:], gt[:, 128 * k:128 * (k + 1)], ident[:])
                nc.scalar.copy(out=gT[:, k, :], in_=tp[:])

            # s1 = src + attn_out: src chunks land via idstack matmuls
            s1 = pps.tile([P, C + 1], F32, tag="ps")
            for k in range(2):
                nc.tensor.matmul(s1[:], srcT[:, k, t * P:(t + 1) * P],
                                 idstack[:, k, :], start=(k == 0), stop=False,
                                 skip_group_check=True)
            nc.tensor.matmul(s1[:], gT[:, 0, :], wout[:, 0, :], start=False,
                             stop=False, skip_group_check=True)
            nc.tensor.matmul(s1[:], gT[:, 1, :], wout[:, 1, :], start=False,
                             stop=True, skip_group_check=True)
            if nz.get("b_out"):
                nc.vector.tensor_tensor(out=s1[:, 0:C], in0=s1[:, 0:C],
                                        in1=cext["b_out"][:], op=OP.add)

            # x1 = LN1(s1)
            x1 = work.tile([P, C], F32, tag="x1")
            _layernorm(nc, work, x1, s1, cext.get("ln1_g"), cext.get("ln1_b"), epsb)

            # FFN
            x1T = work.tile([P, 2, P], BF16, tag="x1T")
            for k in range(2):
                tp = ppt.tile([P, P], F32, tag="pt")
                nc.tensor.transpose(tp[:], x1[:, 128 * k:128 * (k + 1)], ident[:])
                nc.scalar.copy(out=x1T[:, k, :], in_=tp[:])
            hT = work.tile([P, 8, P], BF16, tag="hT")
            for j in range(8):
                fp = ppf.tile([P, P], F32, tag="pf")
                nc.tensor.matmul(fp[:], w1[:, 0, 128 * j:128 * (j + 1)], x1T[:, 0, :],
                                 start=True, stop=False)
                nc.tensor.matmul(fp[:], w1[:, 1, 128 * j:128 * (j + 1)], x1T[:, 1, :],
                                 start=False, stop=True)
                nc.scalar.activation(out=hT[:, j, :], in_=fp[:], func=AF.Relu,
                                     bias=b1t[:, j:j + 1])
            # s2 = x1 + ffn: FFN2 matmuls then x1T idstack-accumulate
            s2 = pps.tile([P, C + 1], F32, tag="ps")
            for j in range(8):
                nc.tensor.matmul(s2[:], hT[:, j, :], w2[:, j, :],
                                 start=(j == 0), stop=False,
                                 skip_group_check=True)
            for k in range(2):
                nc.tensor.matmul(s2[:], x1T[:, k, :], idstack[:, k, :],
                                 start=False, stop=(k == 1),
                                 skip_group_check=True)
            if nz.get("b2"):
                nc.vector.tensor_tensor(out=s2[:, 0:C], in0=s2[:, 0:C],
                                        in1=cext["b2"][:], op=OP.add)
            o_t = io.tile([P, C], BF16, tag="o_t")
            _layernorm(nc, work, o_t, s2, cext.get("ln2_g"), cext.get("ln2_b"), epsb)
            nc.sync.dma_start(out=out_d[r0:r0 + P, :], in_=o_t[:])

        # ---- software-pipelined emission: A(0) | B(0)+A(1) | B(1) ----
        setup_batch(0)
        for tg in range(NG):
            emit_A_group(0, tg)
        emit_idxload(0)
        setup_batch(1)
        for t in range(NT):
            emit_B(0, t)
            if t % 2 == 1 and t // 2 < NG:
                emit_A_group(1, t // 2)
            if t == 2 * NG + 1:
                emit_idxload(1)
        for t in range(NT):
            emit_B(1, t)

    nc.compile()
    return nc


def _layernorm(nc, work, out_t, s, g_rep, b_rep, epsb):
    """s: PSUM [P, 257]; col 256 = mean(s[:, 0:256]) from the matmul mean
    column. Variance = E[x^2] - mean^2 (means here are ~0: no cancellation);
    E[x^2] accumulates on the Activation engine via Square+accum_out."""
    x = s[:, 0:C]
    mean = s[:, C:C + 1]
    sqo = work.tile([P, C], F32, tag="ln_sq")
    ssq = work.tile([P, 1], F32, tag="ln_ssq")
    nc.scalar.activation(out=sqo[:], in_=x, func=AF.Square, accum_out=ssq[:])
    m2 = work.tile([P, 1], F32, tag="ln_m2")
    nc.scalar.activation(out=m2[:], in_=mean, func=AF.Square)
    var = work.tile([P, 1], F32, tag="ln_var")
    nc.vector.scalar_tensor_tensor(out=var[:], in0=ssq[:], scalar=1.0 / C,
                                   in1=m2[:], op0=OP.mult, op1=OP.subtract)
    rstd = work.tile([P, 1], F32, tag="ln_rstd")
    nc.vector.tensor_scalar(out=rstd[:], in0=var[:], scalar1=1e-5,
                            scalar2=None, op0=OP.add)
    nc.vector.reciprocal(out=rstd[:], in_=rstd[:])
    nc.scalar.activation(out=rstd[:], in_=rstd[:], func=AF.Sqrt)
    negmr = work.tile([P, 1], F32, tag="ln_negmr")
    nc.vector.scalar_tensor_tensor(out=negmr[:], in0=mean, scalar=-1.0,
                                   in1=rstd[:], op0=OP.mult, op1=OP.mult)
    if g_rep is None and b_rep is None:
        nc.scalar.activation(out=out_t[:], in_=x, func=AF.Identity,
                             bias=negmr[:, 0:1], scale=rstd[:, 0:1])
        return
    xn = work.tile([P, C], F32, tag="ln_xn")
    nc.scalar.activation(out=xn[:], in_=x, func=AF.Identity,
                         bias=negmr[:, 0:1], scale=rstd[:, 0:1])
    if g_rep is not None and b_rep is not None:
        nc.vector.tensor_tensor(out=xn[:], in0=xn[:], in1=g_rep[:], op=OP.mult)
        nc.vector.tensor_tensor(out=out_t[:], in0=xn[:], in1=b_rep[:], op=OP.add)
    elif g_rep is not None:
        nc.vector.tensor_tensor(out=out_t[:], in0=xn[:], in1=g_rep[:], op=OP.mult)
    else:
        nc.vector.tensor_tensor(out=out_t[:], in0=xn[:], in1=b_rep[:], op=OP.add)


def make_in_maps(inputs):
    import ml_dtypes
    src = np.ascontiguousarray(np.asarray(inputs["src"], dtype=np.float32))
    pos = np.asarray(inputs["pos"], dtype=np.float32)
    ref = np.asarray(inputs["reference_points"], dtype=np.float32)[:, :, 0, :]
    w = {}
    for n in ("W1", "W_off", "W_attn", "W_val"):
        w[n] = np.ascontiguousarray(
            np.asarray(inputs[n], dtype=np.float32).astype(ml_dtypes.bfloat16))
    # W_out / W2 gain a 257th column = row-mean/... the LN mean columns
    for n in ("W_out", "W2"):
        wf = np.asarray(inputs[n], dtype=np.float32)
        wx = np.concatenate([wf, wf.mean(axis=1, keepdims=True)], axis=1)
        w[n] = np.ascontiguousarray(wx.astype(ml_dtypes.bfloat16))
    w["b1"] = np.ascontiguousarray(np.asarray(inputs["b1"], dtype=np.float32))

    # pos@[W_off|W_attn] (+ biases), with the pixel-space reference point
    # folded into the offset lanes: px = src@W_off + posoa = loc*64 - 0.5
    w_off = np.asarray(inputs["W_off"], dtype=np.float32)
    w_attn = np.asarray(inputs["W_attn"], dtype=np.float32)
    posoa = np.empty((B, LQ, 96), dtype=np.float32)
    pos2 = pos.reshape(B * LQ, C)
    posoa[:, :, 0:64] = (pos2 @ w_off + np.asarray(inputs["b_off"], np.float32)
                         ).reshape(B, LQ, 64)
    posoa[:, :, 0:64] += np.tile(ref * 64.0 - 0.5, (1, 1, 32))
    posoa[:, :, 64:96] = (pos2 @ w_attn + np.asarray(inputs["b_attn"], np.float32)
                          ).reshape(B, LQ, 32)

    in_maps = []
    for c in range(NCORES):
        m = dict(w)
        sc = src[BPC * c:BPC * (c + 1)]
        m["srcT"] = np.ascontiguousarray(
            sc.transpose(0, 2, 1).astype(ml_dtypes.bfloat16)).reshape(BPC * C, LQ)
        # regroup to [b, tile-group, q-within-tile, tile-in-group, 96] so
        # each SBUF partition's group slice is one contiguous DMA chunk
        pg = posoa[BPC * c:BPC * (c + 1)].reshape(BPC, LQ // (4 * P), 4, P, 96)
        m["posoa"] = np.ascontiguousarray(
            pg.transpose(0, 1, 3, 2, 4)).reshape(BPC * LQ, 96)
        in_maps.append(m)
    return in_maps


def assemble_output(results):
    out = np.stack([np.asarray(results[c]["out"], dtype=np.float32
                               ).reshape(BPC, LQ, C) for c in range(NCORES)])
    return out.reshape(B, LQ, C)


def kernel(**inputs):
    if "nc" not in _CACHE:
        _CACHE["nc"] = build_nc({k: inputs[k] for k in
                                 ("b_val", "b_out", "b2",
                                  "ln1_g", "ln1_b", "ln2_g", "ln2_b")})
    nc = _CACHE["nc"]
    in_maps = make_in_maps(inputs)
    res = run_bass_kernel_spmd(nc, in_maps, core_ids=list(range(NCORES)))
    return assemble_output(res.results)
